# revision 13
# baseline (speedup 1.0000x reference)
"""Trainium2 Bass kernel for nn_ArchitectureController (sampling).

The LSTM state depends only on the discrete-choice history, so the 13-step
recurrence collapses to a path tree (540 leaves for the default mask). The
tree runs on-device (continuous part sharded over the 8 cores + AllGather of
the per-path Normal stats); the per-sample phase is table gathers (gpsimd
indirect_copy) + small elementwise work, data-parallel over the batch.
"""
import sys
import numpy as np

sys.path.insert(0, "/opt/trn_rl_repo")
import concourse.bass as bass                      # noqa: E402
import concourse.mybir as mybir                    # noqa: E402
from concourse.tile import TileContext             # noqa: E402
from concourse.bass_utils import run_bass_kernel_spmd  # noqa: E402

F32 = mybir.dt.float32
U16 = mybir.dt.uint16
I32 = mybir.dt.int32
AF = mybir.ActivationFunctionType
OP = mybir.AluOpType
AX = mybir.AxisListType

H = 128
ND = 5
NCC = 8
CMAX = 5
NEG = -1e30
SHIFT = 64.0
NCORE = 8
BS = 8192
ROUND_IDX = (0, 1, 2, 5)
C2PI = float(0.5 * np.log(2.0 * np.pi))
DEBUG = False


def _ap(t, pdims, fdims, offset=0):
    """AP on an SBUF tile: pdims in (partition-step, num); fdims in elements."""
    base = t[:] if not isinstance(t, bass.AP) else t
    ps = base.ap[0][0]
    pd = [[s * ps, n] for s, n in pdims]
    return bass.AP(tensor=base.tensor, offset=base.offset + offset,
                   ap=pd + [list(d) for d in fdims])


def _apd(t, dims, offset=0):
    """AP on a DRAM tensor with raw element strides."""
    base = t[:] if not isinstance(t, bass.AP) else t
    return bass.AP(tensor=base.tensor, offset=base.offset + offset,
                   ap=[list(d) for d in dims])


def _perm_chain(counts):
    """Sample layout chain (identical on every core). Returns [(P,M)]*5 and
    the final continuous-gather (group, column) mapping."""
    s = np.arange(BS)
    P = 16 * (s // 1024) + s % 16
    M = (s % 1024) // 16
    chain = []
    for t in range(ND):
        if t > 0:
            g = P // 16
            i = 16 * M + (P % 16)
            P = 32 * (g // 2) + (i % 32)
            M = 2 * (i // 32) + (g % 2)
        chain.append((P.copy(), M.copy()))
    gc = P // 16
    ic = 16 * M + (P % 16)
    return chain, (gc, ic)


def _sizes(counts):
    n_t = [1]
    for k in counts:
        n_t.append(n_t[-1] * int(k))
    nL = n_t[ND]
    LPC = (nL + NCORE - 1) // NCORE
    return n_t, nL, LPC, LPC * NCORE


def _build(counts):
    counts = [int(k) for k in counts]
    n_t, nL, LPC, LPC8 = _sizes(counts)
    nIo = max(nL, 8)

    nc = bass.Bass()
    P = nc.declare_dram_parameter

    wih = P("wih", [128, 2, 512], F32, isOutput=False)
    whh = P("whh", [128, 2, 512], F32, isOutput=False)
    bias2 = P("bias2", [1, 512], F32, isOutput=False)
    embT = P("embT", [128, 26], F32, isOutput=False)
    bias1 = P("bias1", [1, 512], F32, isOutput=False)
    dhwT = P("dhwT", [128, 5, 5], F32, isOutput=False)
    dhb = P("dhb", [1, 5, 5], F32, isOutput=False)
    chwT = P("chwT", [128, 8, 2], F32, isOutput=False)
    chb = P("chb", [1, 8, 2], F32, isOutput=False)
    gum = P("gum", [ND, 128, 64, 5], F32, isOutput=False)
    eps = P("eps", [8, 8, 1024], F32, isOutput=False)
    cR = [P(f"R{t}", [n_t[t], n_t[t + 1]], F32, isOutput=False)
          for t in range(ND - 1)]
    cSel = [P(f"SelT{t}", [26, n_t[t]], F32, isOutput=False) for t in range(ND)]
    cI = P("I128", [128, 128], F32, isOutput=False)
    cOnes = P("ones128", [1, 128], F32, isOutput=False)
    cIota = P("iotaL", [1, nIo], F32, isOutput=False)
    cRev = P("rev320", [1, 320], F32, isOutput=False)
    cBdA = P("bdA", [128, 8], F32, isOutput=False)
    cBdB = P("bdB", [128, 8], F32, isOutput=False)
    cMin = P("mincol", [128, 1], F32, isOutput=False)
    cRng = P("rangecol", [128, 1], F32, isOutput=False)
    cMsk = P("maskcol", [128, 1], F32, isOutput=False)
    cSk = P("Sk", [n_t[ND - 1], LPC], F32, isOutput=False)
    cSelC = P("SelTC", [26, LPC], F32, isOutput=False)

    o_choice = P("choices_raw", [ND, 128, 64], F32, isOutput=True)
    o_params = P("params_raw", [8, 8, 1024], F32, isOutput=True)
    o_lp = P("lp_raw", [8, 1024], F32, isOutput=True)
    if DEBUG:
        o_dbg_logp = P("dbg_logp", [ND, 5, n_t[ND - 1]], F32, isOutput=True)
        o_dbg_lpd = P("dbg_lpd", [1, LPC8], F32, isOutput=True)
        o_dbg_stats = P("dbg_stats", [16, LPC], F32, isOutput=True)
        o_dbg_x5 = P("dbg_x5", [128, 64], F32, isOutput=True)

    import contextlib
    with TileContext(nc) as tc, contextlib.ExitStack() as ctx:
        sb = ctx.enter_context(tc.tile_pool(name="sb", bufs=1))
        sb2 = ctx.enter_context(tc.tile_pool(name="sb2", bufs=2))
        ps = ctx.enter_context(tc.tile_pool(name="ps", bufs=4, space="PSUM"))
        psl = ctx.enter_context(tc.tile_pool(name="psl", bufs=1, space="PSUM"))
        psg = ctx.enter_context(tc.tile_pool(name="psg", bufs=2, space="PSUM"))
        dram = ctx.enter_context(tc.tile_pool(name="dram", bufs=1, space="DRAM"))
        cb = ctx.enter_context(tc.tile_pool(name="cb", bufs=1))
        dma = nc.sync.dma_start
        act = nc.scalar.activation

        def load(par, shape, dtype=F32, tag=None):
            t = sb.tile(shape, dtype, tag=tag or par.name, name=tag or par.name)
            dma(out=t[:], in_=par[:])
            return t

        t_wih = load(wih, [128, 2, 512])
        t_whh = load(whh, [128, 2, 512])
        t_b1 = load(bias1, [1, 512])
        t_b2 = load(bias2, [1, 512])
        t_embT = load(embT, [128, 26])
        t_dhwT = load(dhwT, [128, 5, 5])
        t_dhb = load(dhb, [1, 5, 5])
        t_chwT = load(chwT, [128, 8, 2])
        t_chb = load(chb, [1, 8, 2])
        t_R = [load(cR[t], [n_t[t], n_t[t + 1]], tag=f"R{t}")
               for t in range(ND - 1)]
        t_Sel = [load(cSel[t], [26, n_t[t]], tag=f"Sel{t}") for t in range(ND)]
        t_I = load(cI, [128, 128])
        t_ones = load(cOnes, [1, 128])
        t_bdA = load(cBdA, [128, 8])
        t_bdB = load(cBdB, [128, 8])
        t_min = load(cMin, [128, 1])
        t_rng = load(cRng, [128, 1])
        t_msk = load(cMsk, [128, 1])
        t_Sk = load(cSk, [n_t[ND - 1], LPC])
        t_SelC = load(cSelC, [26, LPC])
        t_rev = sb.tile([128, 64, 5], F32, tag="rev", name="rev")
        dma(out=t_rev[:], in_=_apd(cRev, [[0, 128], [1, 64 * 5]]))

        t_gum = [sb.tile([128, 64, 5], F32, tag=f"gum{t}", name=f"gum{t}") for t in range(ND)]
        for t in range(ND):
            dma(out=t_gum[t][:], in_=gum[t])
        t_eps = sb.tile([128, 1024], F32, tag="eps", name="eps")
        for t in range(8):
            dma(out=_ap(t_eps, [[16, 8]], [[1, 1024]], offset=t * 1024),
                in_=_apd(eps, [[1024, 8], [1, 1024]], offset=t * 8 * 1024))

        # xembg table [26, 512] = embT.T @ wih[0] + bias1 (row 25 = bias1)
        p_xe = ps.tile([26, 512], F32, tag="pt", name="pt")
        nc.tensor.matmul(p_xe[:], t_embT[:], t_wih[:, 0, :], start=True, stop=False)
        nc.tensor.matmul(p_xe[:], t_ones[:, 0:26], t_b1[:], start=False,
                         stop=True)
        t_xe = sb.tile([26, 512], F32, tag="xe", name="xe")
        act(t_xe[:], p_xe[:], AF.Copy)

        # DRAM staging
        stage5 = [dram.tile([5, n_t[t]], F32, tag=f"st5_{t}", name=f"st5_{t}") for t in range(ND)]
        lpk_stage = [dram.tile([n_t[t], counts[t]], F32, tag=f"lpk_{t}", name=f"lpk_{t}")
                     for t in range(ND)]
        lpd_stage = dram.tile([1, LPC8], F32)
        ag_in = dram.tile([16, LPC], F32)
        ag_out = dram.tile([128, LPC], F32)

        def transpose_sb(h_tile, n, tag):
            pT = ps.tile([128, n], F32, tag="pt", name="pt")
            nc.tensor.matmul(pT[:], h_tile[:], t_I[0:n, 0:n], start=True,
                             stop=True)
            sT = sb2.tile([128, n], F32, tag="sT_" + tag, name="sT_" + tag)
            act(sT[:], pT[:], AF.Copy)
            return sT

        def cell(n, hT_sbuf, cE_ap, l, xmm, tagp):
            g = psg.tile([n, 512], F32, tag="gates", name="gates")
            nc.tensor.matmul(g[:], hT_sbuf, t_whh[:, l, :], start=True, stop=False)
            xmm(g)
            sig = sb2.tile([n, 384], F32, tag="sig" + tagp, name="sig" + tagp)
            act(sig[:], g[:, 0:384], AF.Sigmoid)
            tg = sb2.tile([n, 128], F32, tag="tg" + tagp, name="tg" + tagp)
            act(tg[:], g[:, 384:512], AF.Tanh)
            t1 = sb2.tile([n, 128], F32, tag="t1" + tagp, name="t1" + tagp)
            nc.gpsimd.tensor_tensor(t1[:], sig[:, 0:128], tg[:], OP.mult)
            t2 = sb2.tile([n, 128], F32, tag="t2" + tagp, name="t2" + tagp)
            nc.vector.tensor_tensor(t2[:], sig[:, 128:256], cE_ap, OP.mult)
            c_new = sb2.tile([n, 128], F32, tag="cn" + tagp, name="cn" + tagp)
            nc.vector.tensor_tensor(c_new[:], t1[:], t2[:], OP.add)
            tch = sb2.tile([n, 128], F32, tag="tc" + tagp, name="tc" + tagp)
            act(tch[:], c_new[:], AF.Tanh)
            h_new = sb2.tile([n, 128], F32, tag="hn" + tagp, name="hn" + tagp)
            nc.vector.tensor_tensor(h_new[:], sig[:, 256:384], tch[:], OP.mult)
            return h_new, c_new

        # ---------------- tree + per-sample: discrete steps ----------------
        hT_prev = [None, None]   # [128, n] SBUF (transposed states)
        c_prev = [None, None]    # [n, 128] SBUF
        for l in range(2):
            hT_prev[l] = sb.tile([128, 1], F32, tag=f"hT0_{l}", name=f"hT0_{l}")
            c_prev[l] = sb.tile([1, 128], F32, tag=f"c0_{l}", name=f"c0_{l}")
            nc.vector.memset(hT_prev[l][:], 0.0)
            nc.vector.memset(c_prev[l][:], 0.0)

        X = sb.tile([128, 64], U16, tag="X1", name="X1")
        t_lpP = [None] * ND
        h_last = None

        for t in range(ND):
            n = n_t[t]
            K = counts[t]

            def xmm1(g, _t=t, _n=n):
                nc.tensor.matmul(g[:], t_Sel[_t][:], t_xe[:], start=False,
                                 stop=True)
            h1n, c1n = cell(n, hT_prev[0][:], c_prev[0][:], 0, xmm1, "a")
            hT1n = transpose_sb(h1n, n, "h1")

            def xmm2(g, _n=n, _hT1n=hT1n):
                nc.tensor.matmul(g[:], _hT1n[:], t_wih[:, 1, :], start=False,
                                 stop=False)
                nc.tensor.matmul(g[:], t_ones[:, 0:_n], t_b2[:], start=False,
                                 stop=True)
            h2n, c2n = cell(n, hT_prev[1][:], c_prev[1][:], 1, xmm2, "b")
            hT2n = transpose_sb(h2n, n, "h2")

            # logits -> logp
            pl = ps.tile([n, 5], F32, tag="pt", name="pt")
            nc.tensor.matmul(pl[:], hT2n[:], t_dhwT[:, t, :], start=True, stop=False)
            nc.tensor.matmul(pl[:], t_ones[:, 0:n], t_dhb[:, t, :],
                             start=False, stop=True)
            if K < CMAX:
                nc.vector.memset(pl[:, K:CMAX], NEG)
            m = sb2.tile([n, 1], F32, tag="lsm", name="lsm")
            nc.vector.tensor_reduce(m[:], pl[:], AX.X, OP.max)
            negm = sb2.tile([n, 1], F32, tag="negm", name="negm")
            nc.vector.tensor_scalar(negm[:], m[:], -1.0, None, OP.mult)
            e5 = sb2.tile([n, CMAX], F32, tag="e5", name="e5")
            esum = sb2.tile([n, 1], F32, tag="esum", name="esum")
            act(e5[:], pl[:], AF.Exp, bias=negm[:], scale=1.0, accum_out=esum[:])
            lse = sb2.tile([n, 1], F32, tag="lse", name="lse")
            act(lse[:], esum[:], AF.Ln)
            tot = sb2.tile([n, 1], F32, tag="tot", name="tot")
            nc.vector.tensor_tensor(tot[:], m[:], lse[:], OP.add)
            logp = sb2.tile([n, CMAX], F32, tag="logp", name="logp")
            nc.vector.scalar_tensor_tensor(
                logp[:], pl[:], 1.0, _ap(tot, [[1, n]], [[0, CMAX]]),
                OP.mult, OP.subtract)
            # lpPacked staging
            dma(out=lpk_stage[t][:], in_=logp[:, 0:K])
            t_lpP[t] = sb2.tile([1, n * K], F32, tag=f"lpp{t}", name=f"lpp{t}")
            dma(out=t_lpP[t][:], in_=_apd(lpk_stage[t], [[1, 1], [1, n * K]]))
            # shifted/masked table, transposed, to DRAM
            shf = sb2.tile([n, CMAX], F32, tag="shf", name="shf")
            nc.vector.tensor_scalar(shf[:, 0:K], logp[:, 0:K], SHIFT, None,
                                    OP.add)
            if K < CMAX:
                nc.vector.memset(shf[:, K:CMAX], NEG)
            pT5 = ps.tile([CMAX, n], F32, tag="pt", name="pt")
            nc.tensor.matmul(pT5[:], shf[:], t_I[0:n, 0:n], start=True,
                             stop=True)
            sT5 = sb2.tile([CMAX, n], F32, tag="sT5", name="sT5")
            act(sT5[:], pT5[:], AF.Copy)
            dma(out=stage5[t][:], in_=sT5[:])
            if DEBUG:
                dma(out=o_dbg_logp[t, :, 0:n], in_=sT5[:])

            # expansion for next tree step
            if t < ND - 1:
                n_next = n_t[t + 1]
                hT_next, c_next = [None, None], [None, None]
                for l, (hh, cc) in enumerate(((h1n, c1n), (h2n, c2n))):
                    pH = ps.tile([128, n_next], F32, tag="pt", name="pt")
                    nc.tensor.matmul(pH[:], hh[:], t_R[t][:], start=True,
                                     stop=True)
                    hE = sb2.tile([128, n_next], F32, tag=f"hEx{l}", name=f"hEx{l}")
                    act(hE[:], pH[:], AF.Copy)
                    pC = ps.tile([n_next, 128], F32, tag="pt", name="pt")
                    nc.tensor.matmul(pC[:], t_R[t][:], cc[:], start=True,
                                     stop=True)
                    cE = sb2.tile([n_next, 128], F32, tag=f"cEx{l}", name=f"cEx{l}")
                    act(cE[:], pC[:], AF.Copy)
                    hT_next[l], c_next[l] = hE, cE
                hT_prev, c_prev = hT_next, c_next
            else:
                h_last = (h1n, c1n, h2n, c2n)

            # ------------- per-sample discrete step t -------------
            if t == 0:
                l0 = sb2.tile([128, 5], F32, tag="l0", name="l0")
                dma(out=l0[:], in_=_apd(stage5[0], [[0, 128], [1, 5]]))
                pert = sb2.tile([128, 64, 5], F32, tag="pert", name="pert")
                nc.vector.tensor_tensor(
                    pert[:], t_gum[0][:], _ap(l0, [[1, 128]], [[0, 64], [1, 5]]),
                    OP.add)
                id_ap = None
            else:
                ctab = sb2.tile([128, n], F32, tag="ctab", name="ctab")
                for j in range(5):
                    dma(out=_ap(ctab, [[16, 8]], [[1, n]], offset=j * n),
                        in_=_apd(stage5[t], [[0, 8], [1, n]], offset=j * n))
                dma(out=_ap(ctab, [[16, 8]], [[1, n]], offset=5 * n),
                    in_=_apd(cIota, [[0, 8], [1, n]]))
                V = sb2.tile([128, 1024], F32, tag="V", name="V")
                nc.gpsimd.indirect_copy(V[:], ctab[:], X[:], True)
                VT = sb2.tile([128, 1024], F32, tag="VT", name="VT")
                nc.vector.transpose(VT[:], V[:])
                pert = sb2.tile([128, 64, 5], F32, tag="pert", name="pert")
                nc.vector.tensor_tensor(
                    pert[:].rearrange("p (a b) c -> p a b c", a=32),
                    _ap(VT, [[1, 128]], [[32, 32], [16, 2], [1, 5]]),
                    t_gum[t][:].rearrange("p (a b) c -> p a b c", a=32),
                    OP.add)
                id_ap = _ap(VT, [[1, 128]], [[32, 32], [16, 2]], offset=5)
            m5 = sb2.tile([128, 64], F32, tag="m5", name="m5")
            nc.vector.tensor_reduce(
                m5[:].rearrange("p (a b) -> p a b", a=64), pert[:], AX.X,
                OP.max)
            eq = sb2.tile([128, 64, 5], F32, tag="eq", name="eq")
            nc.vector.tensor_tensor(
                eq[:], pert[:], _ap(m5, [[1, 128]], [[1, 64], [0, 5]]),
                OP.is_equal)
            w5 = sb2.tile([128, 64, 5], F32, tag="w5", name="w5")
            nc.gpsimd.tensor_tensor(w5[:], eq[:], t_rev[:], OP.mult)
            cp = sb2.tile([128, 64], F32, tag="cp", name="cp")
            nc.vector.tensor_reduce(
                cp[:].rearrange("p (a b) -> p a b", a=64), w5[:], AX.X,
                OP.max)
            chf = sb2.tile([128, 64], F32, tag="chf", name="chf")
            nc.vector.tensor_scalar(chf[:], cp[:], float(CMAX), -1.0,
                                    OP.subtract, OP.mult)
            dma(out=o_choice[t], in_=chf[:])
            # id update
            last = (t == ND - 1)
            scale = float((2 if last else 1) * K)
            off = float((2 if last else 1) * CMAX)
            csc = -2.0 if last else -1.0
            t1i = sb2.tile([128, 64], F32, tag="t1i", name="t1i")
            if t == 0:
                nc.vector.memset(t1i[:], off)
            else:
                nc.vector.tensor_scalar(
                    t1i[:].rearrange("p (a b) -> p a b", a=32), id_ap, scale,
                    off, OP.mult, OP.add)
            Xn = sb.tile([128, 64], U16, tag=f"X{t + 1}", name=f"X{t + 1}")
            nc.vector.scalar_tensor_tensor(Xn[:], cp[:], csc, t1i[:],
                                           OP.mult, OP.add)
            X = Xn

        # ---------------- LPD chain ----------------
        lpd = t_lpP[0]
        cur = n_t[1]
        for t in range(1, ND):
            K = counts[t]
            nxt = sb2.tile([1, cur * K], F32, tag=f"lpd{t}", name=f"lpd{t}")
            nc.vector.tensor_tensor(
                nxt[:], _ap(lpd, [[1, 1]], [[1, cur], [0, K]]), t_lpP[t][:],
                OP.add)
            lpd = nxt
            cur *= K
        dma(out=lpd_stage[:, 0:nL], in_=lpd[:])
        if DEBUG:
            dma(out=o_dbg_lpd[:, 0:nL], in_=lpd[:])
            xf = sb2.tile([128, 64], F32, tag="xf", name="xf")
            nc.vector.tensor_scalar(xf[:], X[:], 1.0, None, OP.mult)
            dma(out=o_dbg_x5[:], in_=xf[:])

        # ---------------- continuous tree (sharded leaves) ----------------
        hT_c, c_c = [None, None], [None, None]
        for l, (hh, cc) in enumerate(((h_last[0], h_last[1]),
                                      (h_last[2], h_last[3]))):
            pT = ps.tile([128, LPC], F32, tag="pt", name="pt")
            nc.tensor.matmul(pT[:], hh[:], t_Sk[:], start=True, stop=True)
            hT_c[l] = sb2.tile([128, LPC], F32, tag=f"hTc{l}", name=f"hTc{l}")
            act(hT_c[l][:], pT[:], AF.Copy)
            pC = ps.tile([LPC, 128], F32, tag="pt", name="pt")
            nc.tensor.matmul(pC[:], t_Sk[:], cc[:], start=True, stop=True)
            c_c[l] = sb2.tile([LPC, 128], F32, tag=f"ccc{l}", name=f"ccc{l}")
            act(c_c[l][:], pC[:], AF.Copy)

        statsAll = sb.tile([2, 8, LPC], F32, tag="statsAll", name="statsAll")
        for tt in range(NCC):
            def xmm1c(g):
                nc.tensor.matmul(g[:], t_SelC[:], t_xe[:], start=False,
                                 stop=True)
            h1n, c1n = cell(LPC, hT_c[0][:], c_c[0][:], 0, xmm1c, "ca")
            hT1n = transpose_sb(h1n, LPC, "c1")

            def xmm2c(g, _hT1n=hT1n):
                nc.tensor.matmul(g[:], _hT1n[:], t_wih[:, 1, :], start=False,
                                 stop=False)
                nc.tensor.matmul(g[:], t_ones[:, 0:LPC], t_b2[:], start=False,
                                 stop=True)
            h2n, c2n = cell(LPC, hT_c[1][:], c_c[1][:], 1, xmm2c, "cb")
            hT2n = transpose_sb(h2n, LPC, "c2")
            pS = ps.tile([2, LPC], F32, tag="pt", name="pt")
            nc.tensor.matmul(pS[:], t_chwT[:, tt, :], hT2n[:], start=True, stop=False)
            nc.tensor.matmul(pS[:], t_chb[:, tt, :], t_ones[:, 0:LPC],
                             start=False, stop=True)
            act(statsAll[:, tt, :], pS[:], AF.Copy)
            hT_c = [hT1n, hT2n]
            c_c = [c1n, c2n]

        dma(out=_apd(ag_in, [[LPC, 2], [2 * LPC, 8], [1, LPC]]),
            in_=statsAll[:])
        if DEBUG:
            dma(out=_apd(o_dbg_stats, [[LPC, 2], [2 * LPC, 8], [1, LPC]]),
                in_=statsAll[:])
        nc.gpsimd.collective_compute(
            "AllGather", OP.bypass, replica_groups=[list(range(NCORE))],
            ins=[ag_in[:].opt()], outs=[ag_out[:].opt()])

        # ---------------- continuous per-sample ----------------
        conttab = sb.tile([128, LPC8, 2], F32, tag="conttab", name="conttab")
        for t in range(8):
            for m_ in range(2):
                dma(out=_ap(conttab, [[1, 1]], [[2 * LPC, 8], [2, LPC]],
                            offset=t * 2 * LPC8 + m_),
                    in_=_apd(ag_out, [[1, 1], [16 * LPC, 8], [1, LPC]],
                             offset=(2 * t + m_) * LPC))
        dma(out=_ap(conttab, [[1, 1]], [[2, LPC8]], offset=8 * 2 * LPC8),
            in_=_apd(lpd_stage, [[1, 1], [1, LPC8]]))
        for t in range(9):
            dma(out=_ap(conttab, [[16, 8]], [[1, 2 * LPC8]],
                        offset=t * 2 * LPC8),
                in_=_ap(conttab, [[1, 1]], [[0, 8], [1, 2 * LPC8]],
                        offset=t * 2 * LPC8))

        GO = sb.tile([128, 1024, 2], F32, tag="GO", name="GO")
        nc.gpsimd.indirect_copy(GO[:, 0:512, :], conttab[:], X[:, 0:32], True)
        nc.gpsimd.indirect_copy(GO[:, 512:1024, :], conttab[:], X[:, 32:64],
                                True)
        mean_v = _ap(GO, [[1, 128]], [[2, 1024]])
        ls_v = _ap(GO, [[1, 128]], [[2, 1024]], offset=1)

        std = cb.tile([128, 1024], F32, tag="std", name="std")
        act(std[:], ls_v, AF.Exp)
        smp = cb.tile([128, 1024], F32, tag="smp", name="smp")
        nc.vector.tensor_tensor(smp[:], std[:], t_eps[:], OP.mult)
        nc.vector.tensor_tensor(smp[:], smp[:], mean_v, OP.add)
        sg = cb.tile([128, 1024], F32, tag="sg", name="sg")
        act(sg[:], smp[:], AF.Sigmoid)
        pv = cb.tile([128, 1024], F32, tag="pv", name="pv")
        act(pv[:], sg[:], AF.Identity, bias=t_min[:], scale=t_rng[:])
        ri = cb.tile([128, 1024], I32, tag="ri", name="ri")
        act(ri[:], pv[:], AF.Copy)
        rf = cb.tile([128, 1024], F32, tag="rf", name="rf")
        act(rf[:], ri[:], AF.Copy)
        dd = cb.tile([128, 1024], F32, tag="dd", name="dd")
        nc.vector.tensor_tensor(dd[:], rf[:], pv[:], OP.subtract)
        pr = cb.tile([128, 1024], F32, tag="pr", name="pr")
        nc.vector.scalar_tensor_tensor(pr[:], dd[:], t_msk[:], pv[:],
                                       OP.mult, OP.add)
        for t in range(8):
            dma(out=_apd(o_params, [[8 * 1024, 8], [1, 1024]], offset=t * 1024),
                in_=_ap(pr, [[16, 8]], [[1, 1024]], offset=t * 1024))
        lpA = cb.tile([128, 1024], F32, tag="lpA", name="lpA")
        nc.vector.scalar_tensor_tensor(lpA[:], t_eps[:], -0.5, t_eps[:],
                                       OP.mult, OP.mult)
        lpB = cb.tile([128, 1024], F32, tag="lpB", name="lpB")
        nc.vector.scalar_tensor_tensor(lpB[:], ls_v, C2PI, lpA[:],
                                       OP.add, OP.subtract)
        lp_ps = psl.tile([8, 1024], F32, tag="lpps", name="lpps")
        for hf in range(2):
            sl = slice(512 * hf, 512 * (hf + 1))
            nc.tensor.matmul(lp_ps[:, sl], t_bdA[:], lpB[:, sl], start=True,
                             stop=False)
            nc.tensor.matmul(lp_ps[:, sl], t_bdB[:],
                             _ap(GO, [[1, 128]], [[2, 512]],
                                 offset=2 * 512 * hf),
                             start=False, stop=True)
        lp_sb = cb.tile([8, 1024], F32, tag="lpsb", name="lpsb")
        act(lp_sb[:], lp_ps[:], AF.Copy)
        dma(out=o_lp[:], in_=lp_sb[:])

    return nc


def _host_inputs(inputs, counts):
    """Build per-core in_maps + reorder metadata."""
    counts = [int(k) for k in counts]
    n_t, nL, LPC, LPC8 = _sizes(counts)
    emb = np.asarray(inputs["emb"], np.float32)
    dh_w = np.asarray(inputs["dh_w"], np.float32)
    dh_b = np.asarray(inputs["dh_b"], np.float32)
    W_ih = np.asarray(inputs["W_ih"], np.float32)
    W_hh = np.asarray(inputs["W_hh"], np.float32)
    b_ih = np.asarray(inputs["b_ih"], np.float32)
    b_hh = np.asarray(inputs["b_hh"], np.float32)
    ch_w = np.asarray(inputs["ch_w"], np.float32)
    ch_b = np.asarray(inputs["ch_b"], np.float32)
    minv = np.asarray(inputs["min_vals"], np.float32)
    maxv = np.asarray(inputs["max_vals"], np.float32)
    gumbel = np.asarray(inputs["gumbel"], np.float32)
    eps = np.asarray(inputs["eps"], np.float32)

    # gate reorder (i, f, g, o) -> (i, f, o, g)
    perm = np.concatenate([np.arange(0, 256), np.arange(384, 512),
                           np.arange(256, 384)])
    com = {}
    com["wih"] = np.ascontiguousarray(
        np.transpose(W_ih[:, perm, :], (2, 0, 1)))        # [128,2,512]
    com["whh"] = np.ascontiguousarray(
        np.transpose(W_hh[:, perm, :], (2, 0, 1)))
    bsum = (b_ih + b_hh)[:, perm]
    com["bias1"] = bsum[0:1]
    com["bias2"] = bsum[1:2]
    embT = np.zeros((128, 26), np.float32)
    embT[:, :25] = emb.reshape(25, 128).T
    com["embT"] = embT
    com["dhwT"] = np.ascontiguousarray(np.transpose(dh_w, (2, 0, 1)))
    com["dhb"] = dh_b[None]
    com["chwT"] = np.ascontiguousarray(np.transpose(ch_w, (2, 0, 1)))
    com["chb"] = ch_b[None]
    # constants
    for t in range(ND - 1):
        npv, K = n_t[t], counts[t]
        R = np.zeros((npv, npv * K), np.float32)
        for p in range(npv):
            R[p, p * K:(p + 1) * K] = 1.0
        com[f"R{t}"] = R
    for t in range(ND):
        n = n_t[t]
        S = np.zeros((26, n), np.float32)
        if t == 0:
            S[25, 0] = 1.0
        else:
            K = counts[t - 1]
            S[(t - 1) * 5 + (np.arange(n) % K), np.arange(n)] = 1.0
        com[f"SelT{t}"] = S
    com["I128"] = np.eye(128, dtype=np.float32)
    com["ones128"] = np.ones((1, 128), np.float32)
    com["iotaL"] = np.arange(max(nL, 8), dtype=np.float32)[None, :]
    com["rev320"] = np.tile((CMAX - np.arange(CMAX)).astype(np.float32),
                            64)[None, :]
    bdA = np.zeros((128, 8), np.float32)
    bdB = np.zeros((128, 8), np.float32)
    for p in range(128):
        g, t = p // 16, p % 16
        if t < NCC:
            bdA[p, g] = -1.0
        elif t == NCC:
            bdB[p, g] = 1.0
    com["bdA"], com["bdB"] = bdA, bdB
    mn = np.zeros((128, 1), np.float32)
    rg = np.zeros((128, 1), np.float32)
    mk = np.zeros((128, 1), np.float32)
    for p in range(128):
        t = p % 16
        if t < NCC:
            mn[p] = minv[t]
            rg[p] = maxv[t] - minv[t]
            mk[p] = 1.0 if t in ROUND_IDX else 0.0
    com["mincol"], com["rangecol"], com["maskcol"] = mn, rg, mk

    chain, (gc, ic) = _perm_chain(counts)
    # gumbel arrangement per step: arr[t, P, M, j] = gumbel[t, s, j]
    in_maps = []
    K4 = counts[ND - 1]
    n4 = n_t[ND - 1]
    for k in range(NCORE):
        m = dict(com)
        leaves = np.minimum(np.arange(LPC * k, LPC * (k + 1)), nL - 1)
        Sk = np.zeros((n4, LPC), np.float32)
        Sk[leaves // K4, np.arange(LPC)] = 1.0
        SelTC = np.zeros((26, LPC), np.float32)
        SelTC[(ND - 1) * 5 + leaves % K4, np.arange(LPC)] = 1.0
        m["Sk"], m["SelTC"] = Sk, SelTC
        gsh = gumbel[:, k * BS:(k + 1) * BS]    # [5, 8192, 5]
        arr = np.zeros((ND, 128, 64, 5), np.float32)
        for t in range(ND):
            Pt, Mt = chain[t]
            arr[t, Pt, Mt] = gsh[t]
        m["gum"] = arr
        esh = eps[:, k * BS:(k + 1) * BS]       # [8, 8192]
        ear = np.zeros((8, 8, 1024), np.float32)
        ear[:, gc, ic] = esh
        m["eps"] = ear
        in_maps.append(m)
    return in_maps, chain, (gc, ic)


_CACHE = {}


def kernel(emb, dh_w, dh_b, W_ih, W_hh, b_ih, b_hh, ch_w, ch_b,
           min_vals, max_vals, gumbel, eps, choice_mask, batch_size):
    mask = np.asarray(choice_mask)
    counts = tuple(int(c) for c in mask.sum(axis=1))
    for t in range(ND):
        assert mask[t, :counts[t]].all(), "mask must be prefix-form"
    B = int(batch_size)
    assert B == NCORE * BS

    inputs = dict(emb=emb, dh_w=dh_w, dh_b=dh_b, W_ih=W_ih, W_hh=W_hh,
                  b_ih=b_ih, b_hh=b_hh, ch_w=ch_w, ch_b=ch_b,
                  min_vals=min_vals, max_vals=max_vals, gumbel=gumbel,
                  eps=eps)
    if counts not in _CACHE:
        _CACHE[counts] = _build(counts)
    nc = _CACHE[counts]
    in_maps, chain, (gc, ic) = _host_inputs(inputs, counts)
    res = run_bass_kernel_spmd(nc, in_maps, core_ids=list(range(NCORE)))

    choices = np.zeros((ND, B), np.int32)
    params = np.zeros((8, B), np.float32)
    lp = np.zeros(B, np.float32)
    for k in range(NCORE):
        r = res.results[k]
        sl = slice(k * BS, (k + 1) * BS)
        craw = r["choices_raw"]             # [5, 128, 64]
        for t in range(ND):
            Pt, Mt = chain[t]
            choices[t, sl] = np.rint(craw[t, Pt, Mt]).astype(np.int32)
        praw = r["params_raw"]              # [8(g), 8(t), 1024]
        params[:, sl] = praw[gc, :, ic].T
        lp[sl] = r["lp_raw"][gc, ic]
    return choices, params, lp
